# revision 16
# baseline (speedup 1.0000x reference)
"""CorefHead Trainium2 kernel.

Reference computation (B=64, S=512, H=1024, HID=512):
  emb_a = span_mean(bert, offsets[:,0:2])   # [B,H]
  emb_b = span_mean(bert, offsets[:,2:4])   # [B,H]
  emb_p = bert[b, offsets[:,4]]             # [B,H]
  x = concat([emb_a, emb_b, emb_p], -1)     # [B,3H]
  h = leaky_relu(batchnorm_eval(x @ W1 + b1), 0.01)
  out = h @ W2 + b2                         # [B,3]

Strategy: pure data parallel, batch sharded 8 ways (8 batches/core),
DMA-byte and PE-cycle minimized:
  - Host ships ONLY the union of span rows per batch, packed back-to-back
    across the core's 8 batches (no per-batch chunk alignment); the pron
    rows are host-gathered/transposed into a small bf16 block, so they
    never force extra 128-row chunks.
  - mm1 is flipped vs the obvious orientation: the 0/1 span masks
    [128, 16] are the stationary PE operand (LDWEIGHTS of 16 columns is
    ~free) and bert streams as the moving operand at N=512, accumulating
    ALL chunks into one PSUM pair [16, 1024] (cols = 2 spans x 8 slots).
    This cuts PE time ~4x vs loading each bert chunk as weights.
  - x is scaled by 1/span_len in fp32 on the PSUM->SBUF copy, transposed
    to contraction-major via 8 PE transposes, cast to bf16.
  - All small constants ride in TWO packed DMAs (one bf16, one fp32)
    issued first on the ACT ring, so mm1's masks land early.
  - DMA order on the sync ring: bert pieces FIRST, folded W1 (bf16) LAST,
    so mm1 is never starved and mm2 chases the W1 stream.
  - mm2 kc order: pron kcs (16..23, no transpose dependency) first, then
    the 8 transposes, then e0/e1 kcs -- keeps the PE dense (HAM warm)
    instead of ping-ponging with the DVE copies. W1 pieces are shipped
    in the same order mm2 consumes them.
  - Head: per 128-wide hid chunk: PE transpose, +BN bias (folded), leaky
    ReLU, mm3 accumulate [3, 8]; +b2; DMA out.
Host gathers per-core [3, 8] outputs and undoes the batch assignment.
"""

import numpy as np

B, S, H = 64, 512, 1024
HID = 512
EPS = 1e-5
NCORES = 8
BPC = B // NCORES   # batches per core
KC = 3 * H // 128   # 24 contraction chunks for mm2
HC = H // 128       # 8 h-chunks per embedding
NMC = 2 * BPC       # mm1 psum partitions: 2 spans x 8 slots

BERT_PIECE = 3      # bert chunks per DMA piece
# W1 k-chunk pieces, in mm2 consumption order: pron (16..23) then e0/e1;
# small last piece keeps the post-DMA matmul tail short.
W1_PIECES = [(16, 24), (0, 8), (8, 14), (14, 16)]
MM2_ORDER = list(range(16, 24)) + list(range(0, 16))
N_WARM = 10         # PE warm-up dummy matmuls (HAM un-throttle)

# Test-harness hooks (harness calls kernel() with TRACE=False default).
TRACE = False
LAST_RESULT = None

_PROGRAM_CACHE: dict = {}


def _build_program(nch: int):
    """Build + compile the SPMD Bass program for nch 128-row bert chunks."""
    import concourse.bacc as bacc
    import concourse.tile as tile
    import concourse.mybir as mybir
    from concourse.bass import MemorySpace
    from concourse.masks import make_identity

    f32 = mybir.dt.float32
    bf16 = mybir.dt.bfloat16

    nc = bacc.Bacc("TRN2", target_bir_lowering=False, debug=False,
                   num_devices=NCORES)

    # cbf16: [mask (nch*16) | pronT (64)] per partition.
    # cf32:  [sfac (1) | bnbB (32) | w2 (12) | b2 (1)] per partition.
    cbf_d = nc.dram_tensor("cbf", [128, nch * NMC + HC * BPC], bf16,
                           kind="ExternalInput").ap()
    cf32_d = nc.dram_tensor("cf32", [128, 46], f32,
                            kind="ExternalInput").ap()
    bert_d = nc.dram_tensor("bertw", [nch * 128, H], bf16,
                            kind="ExternalInput").ap()
    w1_d = nc.dram_tensor("w1x", [128, KC, HID], bf16,
                          kind="ExternalInput").ap()
    out_d = nc.dram_tensor("out", [3, BPC], f32, kind="ExternalOutput").ap()

    with tile.TileContext(nc) as tc:
        with (
            tc.tile_pool(name="consts", bufs=1) as consts,
            tc.tile_pool(name="bert_pool", bufs=1) as bert_pool,
            tc.tile_pool(name="w1_pool", bufs=1) as w1_pool,
            tc.tile_pool(name="head", bufs=1) as head,
            tc.tile_pool(name="ps_mm1", bufs=1, space=MemorySpace.PSUM) as ps_mm1,
            tc.tile_pool(name="ps_tr", bufs=2, space=MemorySpace.PSUM) as ps_tr,
            tc.tile_pool(name="ps_mm2", bufs=1, space=MemorySpace.PSUM) as ps_mm2,
            tc.tile_pool(name="ps_mm3", bufs=1, space=MemorySpace.PSUM) as ps_mm3,
        ):
            # --- everything on the SP HWDGE ring, consts FIRST so mm1's
            # masks are never starved behind the bert stream ---
            cbf_t = consts.tile([128, nch * NMC + HC * BPC], bf16)
            nc.sync.dma_start(out=cbf_t, in_=cbf_d)
            cf32_t = consts.tile([128, 46], f32)
            nc.sync.dma_start(out=cf32_t, in_=cf32_d)
            mask_t = cbf_t[:, 0:nch * NMC].rearrange(
                "p (c m) -> p c m", c=nch)
            pron_t = cbf_t[:, nch * NMC:].rearrange(
                "p (k s) -> p k s", k=HC)
            sfac_t = cf32_t[0:NMC, 0:1]
            bnbB_t = cf32_t[:, 1:33].rearrange("p (m s) -> p m s", m=4)
            w2_t = cf32_t[:, 33:45].rearrange("p (m j) -> p m j", m=4)
            b2_t = cf32_t[0:3, 45:46]
            idt = consts.tile([NMC, NMC], f32)
            make_identity(nc, idt)

            # --- bulk streams: bert FIRST, W1 LAST ---
            # graduated piece sizes: tiny first piece so mm1 starts ASAP,
            # larger later pieces for wire efficiency (PE lags behind DMA
            # by then anyway).
            bert_t = bert_pool.tile([128, nch, H], bf16)
            bert_src = bert_d.rearrange("(c p) h -> p c h", p=128)
            c0 = 0
            for sz in (1, 2, 3, 3, 4, 4, 8):
                if c0 >= nch:
                    break
                c1 = min(c0 + sz, nch)
                if sz == 8:
                    c1 = nch
                nc.sync.dma_start(out=bert_t[:, c0:c1, :],
                                  in_=bert_src[:, c0:c1, :])
                c0 = c1
            w1_t = w1_pool.tile([128, KC, HID], bf16)
            for k0, k1 in W1_PIECES:
                nc.sync.dma_start(out=w1_t[:, k0:k1, :],
                                  in_=w1_d[:, k0:k1, :])

            # --- mm1: span sums, masks stationary, bert streaming ---
            x_sb = consts.tile([NMC, H], f32)
            ps_lo = ps_mm1.tile([NMC, 512], f32, tag="lo")
            ps_hi = ps_mm1.tile([NMC, 512], f32, tag="hi")
            for ch in range(nch):
                nc.tensor.matmul(ps_lo, mask_t[:, ch, :],
                                 bert_t[:, ch, 0:512],
                                 start=(ch == 0), stop=(ch == nch - 1))
                nc.tensor.matmul(ps_hi, mask_t[:, ch, :],
                                 bert_t[:, ch, 512:H],
                                 start=(ch == 0), stop=(ch == nch - 1))

            # --- mm2 part 1: pron kcs (no transpose dependency) ---
            ph = ps_mm2.tile([BPC, HID], f32)
            for i, kc in enumerate(MM2_ORDER[:HC]):
                nc.tensor.matmul(ph, pron_t[:, kc - 2 * HC, :],
                                 w1_t[:, kc, :], start=(i == 0), stop=False)

            # --- scale by 1/span_len (fp32), transpose to xT, cast bf16 ---
            nc.vector.tensor_scalar_mul(x_sb[:, 0:512], ps_lo, sfac_t)
            nc.vector.tensor_scalar_mul(x_sb[:, 512:H], ps_hi, sfac_t)
            xT_t = consts.tile([128, 2 * HC, BPC], bf16)
            for hc in range(HC):
                pht = ps_tr.tile([128, NMC], f32, tag="pht")
                nc.tensor.transpose(
                    pht, x_sb[:, hc * 128:(hc + 1) * 128], idt)
                nc.vector.tensor_copy(xT_t[:, hc, :], pht[:, 0:BPC])
                nc.vector.tensor_copy(xT_t[:, HC + hc, :], pht[:, BPC:NMC])

            # --- mm2 part 2: span kcs ---
            for i, kc in enumerate(MM2_ORDER[HC:]):
                nc.tensor.matmul(ph, xT_t[:, kc, :], w1_t[:, kc, :],
                                 start=False, stop=(i == 2 * HC - 1))

            # --- head: transpose h (4x into one PSUM tile), then single
            # +BN-bias and LeakyReLU over [128, 4*8], then mm3 ---
            hs_t = head.tile([BPC, HID], f32)
            ot_ps = ps_mm3.tile([3, BPC], f32, tag="oT")
            phtH = ps_tr.tile([128, 4 * BPC], f32, tag="phtH")
            for mc in range(HID // 128):
                nc.vector.tensor_copy(hs_t[:, mc * 128:(mc + 1) * 128],
                                      ph[:, mc * 128:(mc + 1) * 128])
                nc.tensor.transpose(
                    phtH[:, mc * BPC:(mc + 1) * BPC],
                    hs_t[:, mc * 128:(mc + 1) * 128],
                    idt[0:BPC, 0:BPC])
            t_t = head.tile([128, 4, BPC], f32, tag="t_t")
            nc.vector.tensor_add(
                t_t, phtH.rearrange("p (m s) -> p m s", m=4), bnbB_t)
            y_t = head.tile([128, 4, BPC], f32, tag="y_t")
            # y = max(0.01 * t, t)
            nc.vector.scalar_tensor_tensor(
                y_t, t_t, 0.01, t_t,
                op0=mybir.AluOpType.mult, op1=mybir.AluOpType.max)
            for mc in range(HID // 128):
                nc.tensor.matmul(
                    ot_ps, w2_t[:, mc, :], y_t[:, mc, :],
                    start=(mc == 0), stop=(mc == HID // 128 - 1))

            o_t = head.tile([3, BPC], f32)
            nc.vector.tensor_scalar_add(o_t, ot_ps, b2_t)
            nc.scalar.dma_start(out=out_d, in_=o_t)

    nc.compile()
    return nc


def _assign_batches(offs):
    """Union-row counts + balanced assignment of 8 batches to each core.

    Constrained LPT: batches sorted by union size desc, each goes to the
    least-loaded core that still has a free slot.
    """
    urows = np.empty(B, dtype=np.int64)
    for b in range(B):
        a0, a1, b0, b1_, _ = (int(v) for v in offs[b])
        la = a1 - a0 + 1
        lb = b1_ - b0 + 1
        ov = max(0, min(a1, b1_) - max(a0, b0) + 1)
        urows[b] = la + lb - ov
    order = np.argsort(-urows, kind="stable")
    loads = np.zeros(NCORES, dtype=np.int64)
    counts = np.zeros(NCORES, dtype=np.int64)
    assign = [[] for _ in range(NCORES)]
    for b in order:
        free = np.where(counts < BPC)[0]
        c = free[np.argmin(loads[free])]
        assign[c].append(int(b))
        loads[c] += urows[b]
        counts[c] += 1
    nch = int(np.ceil(loads.max() / 128))
    return assign, nch


def _prep_core_inputs(bert_f32, offs, batches, nch, w1x, cf32):
    """Build the per-core input map for the given 8 global batch ids."""
    import ml_dtypes
    bf16 = ml_dtypes.bfloat16

    rows_l, slots_l = [], []
    sfac = np.zeros((128,), dtype=np.float32)
    for s, gb in enumerate(batches):
        a0, a1, b0, b1_, _ = (int(v) for v in offs[gb])
        rows = np.union1d(np.arange(a0, a1 + 1), np.arange(b0, b1_ + 1))
        rows_l.append(rows)
        slots_l.append(np.full(len(rows), s, dtype=np.int64))
        sfac[s] = 1.0 / (a1 - a0 + 1)
        sfac[BPC + s] = 1.0 / (b1_ - b0 + 1)
    rows_cat = np.concatenate(rows_l)
    slots_cat = np.concatenate(slots_l)
    gb_cat = np.asarray(batches)[slots_cat]
    R = len(rows_cat)

    bertw = np.zeros((nch * 128, H), dtype=bf16)
    bertw[:R] = bert_f32[gb_cat, rows_cat].astype(bf16)

    a0s = offs[np.asarray(batches), 0][slots_cat]
    a1s = offs[np.asarray(batches), 1][slots_cat]
    b0s = offs[np.asarray(batches), 2][slots_cat]
    b1s = offs[np.asarray(batches), 3][slots_cat]
    mA = (rows_cat >= a0s) & (rows_cat <= a1s)
    mB = (rows_cat >= b0s) & (rows_cat <= b1s)
    maskflat = np.zeros((nch * 128, NMC), dtype=np.float32)
    idx = np.arange(R)
    maskflat[idx, slots_cat] = mA
    maskflat[idx, BPC + slots_cat] = mB
    maskp = maskflat.reshape(nch, 128, NMC).transpose(1, 0, 2)

    prons = offs[np.asarray(batches), 4]
    pron_rows = bert_f32[np.asarray(batches), prons]  # [BPC, H]
    pronT = pron_rows.reshape(BPC, HC, 128).transpose(2, 1, 0)  # [128,HC,BPC]

    cbf = np.empty((128, nch * NMC + HC * BPC), dtype=bf16)
    cbf[:, 0:nch * NMC] = maskp.reshape(128, nch * NMC).astype(bf16)
    cbf[:, nch * NMC:] = pronT.reshape(128, HC * BPC).astype(bf16)

    cf = cf32.copy()
    cf[:, 0] = sfac

    return {"cbf": cbf, "cf32": cf, "bertw": bertw, "w1x": w1x}


def kernel(bert_outputs, offsets, W1, b1, gamma, beta, running_mean,
           running_var, W2, b2):
    import ml_dtypes

    bert_f32 = np.ascontiguousarray(np.asarray(bert_outputs, dtype=np.float32))
    offs = np.asarray(offsets).astype(np.int64)
    W1 = np.asarray(W1, dtype=np.float32)
    b1 = np.asarray(b1, dtype=np.float32)
    gamma = np.asarray(gamma, dtype=np.float32)
    beta = np.asarray(beta, dtype=np.float32)
    rm = np.asarray(running_mean, dtype=np.float32)
    rv = np.asarray(running_var, dtype=np.float32)
    W2 = np.asarray(W2, dtype=np.float32)
    b2 = np.asarray(b2, dtype=np.float32)

    # Fold BN eval-mode stats: bn(xW1 + b1) = x(W1*s) + ((b1 - mean)*s + beta)
    s = gamma / np.sqrt(rv + EPS)
    bias = (b1 - rm) * s + beta
    w1f = (W1 * s[None, :]).astype(np.float32)
    # [3H, HID] -> [128 p, kc = e*8 + hc, HID], row = e*1024 + hc*128 + p
    w1x = np.ascontiguousarray(
        w1f.reshape(3, HC, 128, HID).transpose(2, 0, 1, 3)
        .reshape(128, KC, HID)).astype(ml_dtypes.bfloat16)

    # packed fp32 consts: [sfac | bnbB (4*8, bias bcast over slots) |
    #                      w2 (12) | b2 (1)]
    cf32 = np.zeros((128, 46), dtype=np.float32)
    cf32[:, 1:33] = np.repeat(
        bias.reshape(HID // 128, 128).T[:, :, None], BPC, axis=2
    ).reshape(128, 32)
    cf32[:, 33:45] = W2.reshape(HID // 128, 128, 3).transpose(1, 0, 2) \
        .reshape(128, 12)
    cf32[0:3, 45] = b2

    assign, nch = _assign_batches(offs)

    if nch not in _PROGRAM_CACHE:
        _PROGRAM_CACHE[nch] = _build_program(nch)
    nc = _PROGRAM_CACHE[nch]

    in_maps = [
        _prep_core_inputs(bert_f32, offs, assign[c], nch, w1x, cf32)
        for c in range(NCORES)
    ]

    from concourse import bass_utils
    kwargs = {}
    if TRACE:
        kwargs = {"trace": True, "trace_cores": list(range(NCORES))}
    res = bass_utils.run_bass_kernel_spmd(nc, in_maps,
                                          core_ids=list(range(NCORES)),
                                          **kwargs)
    global LAST_RESULT
    LAST_RESULT = res

    out = np.empty((B, 3), dtype=np.float32)
    for c in range(NCORES):
        out[assign[c]] = res.results[c]["out"].T
    return out


# revision 18
# speedup vs baseline: 1.0255x; 1.0255x over previous
"""CorefHead Trainium2 kernel.

Reference computation (B=64, S=512, H=1024, HID=512):
  emb_a = span_mean(bert, offsets[:,0:2])   # [B,H]
  emb_b = span_mean(bert, offsets[:,2:4])   # [B,H]
  emb_p = bert[b, offsets[:,4]]             # [B,H]
  x = concat([emb_a, emb_b, emb_p], -1)     # [B,3H]
  h = leaky_relu(batchnorm_eval(x @ W1 + b1), 0.01)
  out = h @ W2 + b2                         # [B,3]

Strategy: pure data parallel, batch sharded 8 ways (8 batches/core),
DMA-byte and PE-cycle minimized:
  - Host ships ONLY the union of span rows per batch, packed back-to-back
    across the core's 8 batches (no per-batch chunk alignment); the pron
    rows are host-gathered/transposed into a small bf16 block, so they
    never force extra 128-row chunks.
  - mm1 is flipped vs the obvious orientation: the 0/1 span masks
    [128, 16] are the stationary PE operand (LDWEIGHTS of 16 columns is
    ~free) and bert streams as the moving operand at N=512, accumulating
    ALL chunks into one PSUM pair [16, 1024] (cols = 2 spans x 8 slots).
    This cuts PE time ~4x vs loading each bert chunk as weights.
  - x is scaled by 1/span_len in fp32 on the PSUM->SBUF copy, transposed
    to contraction-major via 8 PE transposes, cast to bf16.
  - All small constants ride in TWO packed DMAs (one bf16, one fp32)
    issued first on the ACT ring, so mm1's masks land early.
  - DMA order on the sync ring: bert pieces FIRST, folded W1 (bf16) LAST,
    so mm1 is never starved and mm2 chases the W1 stream.
  - mm2 kc order: pron kcs (16..23, no transpose dependency) first, then
    the 8 transposes, then e0/e1 kcs -- keeps the PE dense (HAM warm)
    instead of ping-ponging with the DVE copies. W1 pieces are shipped
    in the same order mm2 consumes them.
  - Head: per 128-wide hid chunk: PE transpose, +BN bias (folded), leaky
    ReLU, mm3 accumulate [3, 8]; +b2; DMA out.
Host gathers per-core [3, 8] outputs and undoes the batch assignment.
"""

import numpy as np

B, S, H = 64, 512, 1024
HID = 512
EPS = 1e-5
NCORES = 8
BPC = B // NCORES   # batches per core
KC = 3 * H // 128   # 24 contraction chunks for mm2
HC = H // 128       # 8 h-chunks per embedding
NMC = 2 * BPC       # mm1 psum partitions: 2 spans x 8 slots

BERT_PIECE = 3      # bert chunks per DMA piece
# W1 k-chunk pieces, in mm2 consumption order: pron (16..23) then e0/e1.
W1_PIECES = [(16, 20), (20, 24), (0, 4), (4, 8), (8, 12), (12, 16)]
MM2_ORDER = list(range(16, 24)) + list(range(0, 16))

# Test-harness hooks (harness calls kernel() with TRACE=False default).
TRACE = False
LAST_RESULT = None

_PROGRAM_CACHE: dict = {}


def _build_program(nch: int):
    """Build + compile the SPMD Bass program for nch 128-row bert chunks."""
    import concourse.bacc as bacc
    import concourse.tile as tile
    import concourse.mybir as mybir
    from concourse.bass import MemorySpace
    from concourse.masks import make_identity

    f32 = mybir.dt.float32
    bf16 = mybir.dt.bfloat16

    nc = bacc.Bacc("TRN2", target_bir_lowering=False, debug=False,
                   num_devices=NCORES)

    # cbf16: [mask (nch*16) | pronT (64)] per partition.
    # cf32:  [sfac (1) | bnbB (32) | w2 (12) | b2 (1)] per partition.
    cbf_d = nc.dram_tensor("cbf", [128, nch * NMC + HC * BPC], bf16,
                           kind="ExternalInput").ap()
    cf32_d = nc.dram_tensor("cf32", [128, 46], f32,
                            kind="ExternalInput").ap()
    bert_d = nc.dram_tensor("bertw", [nch * 128, H], bf16,
                            kind="ExternalInput").ap()
    w1_d = nc.dram_tensor("w1x", [128, KC, HID], bf16,
                          kind="ExternalInput").ap()
    out_d = nc.dram_tensor("out", [3, BPC], f32, kind="ExternalOutput").ap()

    with tile.TileContext(nc) as tc:
        with (
            tc.tile_pool(name="consts", bufs=1) as consts,
            tc.tile_pool(name="bert_pool", bufs=1) as bert_pool,
            tc.tile_pool(name="w1_pool", bufs=1) as w1_pool,
            tc.tile_pool(name="head", bufs=1) as head,
            tc.tile_pool(name="ps_mm1", bufs=1, space=MemorySpace.PSUM) as ps_mm1,
            tc.tile_pool(name="ps_tr", bufs=2, space=MemorySpace.PSUM) as ps_tr,
            tc.tile_pool(name="ps_mm2", bufs=1, space=MemorySpace.PSUM) as ps_mm2,
            tc.tile_pool(name="ps_mm3", bufs=1, space=MemorySpace.PSUM) as ps_mm3,
        ):
            # --- everything on the SP HWDGE ring, consts FIRST so mm1's
            # masks are never starved behind the bert stream ---
            cbf_t = consts.tile([128, nch * NMC + HC * BPC], bf16)
            nc.sync.dma_start(out=cbf_t, in_=cbf_d)
            cf32_t = consts.tile([128, 46], f32)
            nc.sync.dma_start(out=cf32_t, in_=cf32_d)
            mask_t = cbf_t[:, 0:nch * NMC].rearrange(
                "p (c m) -> p c m", c=nch)
            pron_t = cbf_t[:, nch * NMC:].rearrange(
                "p (k s) -> p k s", k=HC)
            sfac_t = cf32_t[0:NMC, 0:1]
            bnbB_t = cf32_t[:, 1:33].rearrange("p (m s) -> p m s", m=4)
            w2_t = cf32_t[:, 33:45].rearrange("p (m j) -> p m j", m=4)
            b2_t = cf32_t[0:3, 45:46]
            idt = consts.tile([NMC, NMC], f32)
            make_identity(nc, idt)

            # --- bulk streams: bert FIRST, W1 LAST ---
            bert_t = bert_pool.tile([128, nch, H], bf16)
            bert_src = bert_d.rearrange("(c p) h -> p c h", p=128)
            for c0 in range(0, nch, BERT_PIECE):
                c1 = min(c0 + BERT_PIECE, nch)
                nc.sync.dma_start(out=bert_t[:, c0:c1, :],
                                  in_=bert_src[:, c0:c1, :])
            w1_t = w1_pool.tile([128, KC, HID], bf16)
            for k0, k1 in W1_PIECES:
                nc.sync.dma_start(out=w1_t[:, k0:k1, :],
                                  in_=w1_d[:, k0:k1, :])

            # --- mm1: span sums, masks stationary, bert streaming ---
            x_sb = consts.tile([NMC, H], f32)
            ps_lo = ps_mm1.tile([NMC, 512], f32, tag="lo")
            ps_hi = ps_mm1.tile([NMC, 512], f32, tag="hi")
            for ch in range(nch):
                nc.tensor.matmul(ps_lo, mask_t[:, ch, :],
                                 bert_t[:, ch, 0:512],
                                 start=(ch == 0), stop=(ch == nch - 1))
                nc.tensor.matmul(ps_hi, mask_t[:, ch, :],
                                 bert_t[:, ch, 512:H],
                                 start=(ch == 0), stop=(ch == nch - 1))

            # --- mm2 part 1: pron kcs (no transpose dependency) ---
            ph = ps_mm2.tile([BPC, HID], f32)
            for i, kc in enumerate(MM2_ORDER[:HC]):
                nc.tensor.matmul(ph, pron_t[:, kc - 2 * HC, :],
                                 w1_t[:, kc, :], start=(i == 0), stop=False)

            # --- scale by 1/span_len (fp32), transpose to xT, cast bf16 ---
            nc.vector.tensor_scalar_mul(x_sb[:, 0:512], ps_lo, sfac_t)
            nc.vector.tensor_scalar_mul(x_sb[:, 512:H], ps_hi, sfac_t)
            xT_t = consts.tile([128, 2 * HC, BPC], bf16)
            for hc in range(HC):
                pht = ps_tr.tile([128, NMC], f32, tag="pht")
                nc.tensor.transpose(
                    pht, x_sb[:, hc * 128:(hc + 1) * 128], idt)
                nc.vector.tensor_copy(xT_t[:, hc, :], pht[:, 0:BPC])
                nc.vector.tensor_copy(xT_t[:, HC + hc, :], pht[:, BPC:NMC])

            # --- mm2 part 2: span kcs ---
            for i, kc in enumerate(MM2_ORDER[HC:]):
                nc.tensor.matmul(ph, xT_t[:, kc, :], w1_t[:, kc, :],
                                 start=False, stop=(i == 2 * HC - 1))

            # --- head: transpose h (4x into one PSUM tile), then single
            # +BN-bias and LeakyReLU over [128, 4*8], then mm3 ---
            hs_t = head.tile([BPC, HID], f32)
            ot_ps = ps_mm3.tile([3, BPC], f32, tag="oT")
            phtH = ps_tr.tile([128, 4 * BPC], f32, tag="phtH")
            for mc in range(HID // 128):
                nc.vector.tensor_copy(hs_t[:, mc * 128:(mc + 1) * 128],
                                      ph[:, mc * 128:(mc + 1) * 128])
                nc.tensor.transpose(
                    phtH[:, mc * BPC:(mc + 1) * BPC],
                    hs_t[:, mc * 128:(mc + 1) * 128],
                    idt[0:BPC, 0:BPC])
            t_t = head.tile([128, 4, BPC], f32, tag="t_t")
            nc.vector.tensor_add(
                t_t, phtH.rearrange("p (m s) -> p m s", m=4), bnbB_t)
            y_t = head.tile([128, 4, BPC], f32, tag="y_t")
            # y = max(0.01 * t, t)
            nc.vector.scalar_tensor_tensor(
                y_t, t_t, 0.01, t_t,
                op0=mybir.AluOpType.mult, op1=mybir.AluOpType.max)
            for mc in range(HID // 128):
                nc.tensor.matmul(
                    ot_ps, w2_t[:, mc, :], y_t[:, mc, :],
                    start=(mc == 0), stop=(mc == HID // 128 - 1))

            o_t = head.tile([3, BPC], f32)
            nc.vector.tensor_scalar_add(o_t, ot_ps, b2_t)
            nc.scalar.dma_start(out=out_d, in_=o_t)

    nc.compile()
    return nc


def _assign_batches(offs):
    """Union-row counts + balanced assignment of 8 batches to each core.

    Constrained LPT: batches sorted by union size desc, each goes to the
    least-loaded core that still has a free slot.
    """
    urows = np.empty(B, dtype=np.int64)
    for b in range(B):
        a0, a1, b0, b1_, _ = (int(v) for v in offs[b])
        la = a1 - a0 + 1
        lb = b1_ - b0 + 1
        ov = max(0, min(a1, b1_) - max(a0, b0) + 1)
        urows[b] = la + lb - ov
    order = np.argsort(-urows, kind="stable")
    loads = np.zeros(NCORES, dtype=np.int64)
    counts = np.zeros(NCORES, dtype=np.int64)
    assign = [[] for _ in range(NCORES)]
    for b in order:
        free = np.where(counts < BPC)[0]
        c = free[np.argmin(loads[free])]
        assign[c].append(int(b))
        loads[c] += urows[b]
        counts[c] += 1
    nch = int(np.ceil(loads.max() / 128))
    return assign, nch


def _prep_core_inputs(bert_f32, offs, batches, nch, w1x, cf32):
    """Build the per-core input map for the given 8 global batch ids."""
    import ml_dtypes
    bf16 = ml_dtypes.bfloat16

    rows_l, slots_l = [], []
    sfac = np.zeros((128,), dtype=np.float32)
    for s, gb in enumerate(batches):
        a0, a1, b0, b1_, _ = (int(v) for v in offs[gb])
        rows = np.union1d(np.arange(a0, a1 + 1), np.arange(b0, b1_ + 1))
        rows_l.append(rows)
        slots_l.append(np.full(len(rows), s, dtype=np.int64))
        sfac[s] = 1.0 / (a1 - a0 + 1)
        sfac[BPC + s] = 1.0 / (b1_ - b0 + 1)
    rows_cat = np.concatenate(rows_l)
    slots_cat = np.concatenate(slots_l)
    gb_cat = np.asarray(batches)[slots_cat]
    R = len(rows_cat)

    bertw = np.zeros((nch * 128, H), dtype=bf16)
    bertw[:R] = bert_f32[gb_cat, rows_cat].astype(bf16)

    a0s = offs[np.asarray(batches), 0][slots_cat]
    a1s = offs[np.asarray(batches), 1][slots_cat]
    b0s = offs[np.asarray(batches), 2][slots_cat]
    b1s = offs[np.asarray(batches), 3][slots_cat]
    mA = (rows_cat >= a0s) & (rows_cat <= a1s)
    mB = (rows_cat >= b0s) & (rows_cat <= b1s)
    maskflat = np.zeros((nch * 128, NMC), dtype=np.float32)
    idx = np.arange(R)
    maskflat[idx, slots_cat] = mA
    maskflat[idx, BPC + slots_cat] = mB
    maskp = maskflat.reshape(nch, 128, NMC).transpose(1, 0, 2)

    prons = offs[np.asarray(batches), 4]
    pron_rows = bert_f32[np.asarray(batches), prons]  # [BPC, H]
    pronT = pron_rows.reshape(BPC, HC, 128).transpose(2, 1, 0)  # [128,HC,BPC]

    cbf = np.empty((128, nch * NMC + HC * BPC), dtype=bf16)
    cbf[:, 0:nch * NMC] = maskp.reshape(128, nch * NMC).astype(bf16)
    cbf[:, nch * NMC:] = pronT.reshape(128, HC * BPC).astype(bf16)

    cf = cf32.copy()
    cf[:, 0] = sfac

    return {"cbf": cbf, "cf32": cf, "bertw": bertw, "w1x": w1x}


def kernel(bert_outputs, offsets, W1, b1, gamma, beta, running_mean,
           running_var, W2, b2):
    import ml_dtypes

    bert_f32 = np.ascontiguousarray(np.asarray(bert_outputs, dtype=np.float32))
    offs = np.asarray(offsets).astype(np.int64)
    W1 = np.asarray(W1, dtype=np.float32)
    b1 = np.asarray(b1, dtype=np.float32)
    gamma = np.asarray(gamma, dtype=np.float32)
    beta = np.asarray(beta, dtype=np.float32)
    rm = np.asarray(running_mean, dtype=np.float32)
    rv = np.asarray(running_var, dtype=np.float32)
    W2 = np.asarray(W2, dtype=np.float32)
    b2 = np.asarray(b2, dtype=np.float32)

    # Fold BN eval-mode stats: bn(xW1 + b1) = x(W1*s) + ((b1 - mean)*s + beta)
    s = gamma / np.sqrt(rv + EPS)
    bias = (b1 - rm) * s + beta
    w1f = (W1 * s[None, :]).astype(np.float32)
    # [3H, HID] -> [128 p, kc = e*8 + hc, HID], row = e*1024 + hc*128 + p
    w1x = np.ascontiguousarray(
        w1f.reshape(3, HC, 128, HID).transpose(2, 0, 1, 3)
        .reshape(128, KC, HID)).astype(ml_dtypes.bfloat16)

    # packed fp32 consts: [sfac | bnbB (4*8, bias bcast over slots) |
    #                      w2 (12) | b2 (1)]
    cf32 = np.zeros((128, 46), dtype=np.float32)
    cf32[:, 1:33] = np.repeat(
        bias.reshape(HID // 128, 128).T[:, :, None], BPC, axis=2
    ).reshape(128, 32)
    cf32[:, 33:45] = W2.reshape(HID // 128, 128, 3).transpose(1, 0, 2) \
        .reshape(128, 12)
    cf32[0:3, 45] = b2

    assign, nch = _assign_batches(offs)

    if nch not in _PROGRAM_CACHE:
        _PROGRAM_CACHE[nch] = _build_program(nch)
    nc = _PROGRAM_CACHE[nch]

    in_maps = [
        _prep_core_inputs(bert_f32, offs, assign[c], nch, w1x, cf32)
        for c in range(NCORES)
    ]

    from concourse import bass_utils
    kwargs = {}
    if TRACE:
        kwargs = {"trace": True, "trace_cores": list(range(NCORES))}
    res = bass_utils.run_bass_kernel_spmd(nc, in_maps,
                                          core_ids=list(range(NCORES)),
                                          **kwargs)
    global LAST_RESULT
    LAST_RESULT = res

    out = np.empty((B, 3), dtype=np.float32)
    for c in range(NCORES):
        out[assign[c]] = res.results[c]["out"].T
    return out


# revision 20
# speedup vs baseline: 1.0593x; 1.0329x over previous
"""CorefHead Trainium2 kernel.

Reference computation (B=64, S=512, H=1024, HID=512):
  emb_a = span_mean(bert, offsets[:,0:2])   # [B,H]
  emb_b = span_mean(bert, offsets[:,2:4])   # [B,H]
  emb_p = bert[b, offsets[:,4]]             # [B,H]
  x = concat([emb_a, emb_b, emb_p], -1)     # [B,3H]
  h = leaky_relu(batchnorm_eval(x @ W1 + b1), 0.01)
  out = h @ W2 + b2                         # [B,3]

Strategy: pure data parallel, batch sharded 8 ways (8 batches/core),
DMA-byte and PE-cycle minimized:
  - Host ships ONLY the union of span rows per batch, packed back-to-back
    across the core's 8 batches (no per-batch chunk alignment); the pron
    rows are host-gathered/transposed into a small bf16 block, so they
    never force extra 128-row chunks.
  - mm1 is flipped vs the obvious orientation: the 0/1 span masks
    [128, 16] are the stationary PE operand (LDWEIGHTS of 16 columns is
    ~free) and bert streams as the moving operand at N=512, accumulating
    ALL chunks into one PSUM pair [16, 1024] (cols = 2 spans x 8 slots).
    This cuts PE time ~4x vs loading each bert chunk as weights.
  - x is scaled by 1/span_len in fp32 on the PSUM->SBUF copy, transposed
    to contraction-major via 8 PE transposes, cast to bf16.
  - All small constants ride in TWO packed DMAs (one bf16, one fp32)
    issued first on the ACT ring, so mm1's masks land early.
  - DMA order on the sync ring: bert pieces FIRST, folded W1 (bf16) LAST,
    so mm1 is never starved and mm2 chases the W1 stream.
  - mm2 kc order: pron kcs (16..23, no transpose dependency) first, then
    the 8 transposes, then e0/e1 kcs -- keeps the PE dense (HAM warm)
    instead of ping-ponging with the DVE copies. W1 pieces are shipped
    in the same order mm2 consumes them.
  - Head: per 128-wide hid chunk: PE transpose, +BN bias (folded), leaky
    ReLU, mm3 accumulate [3, 8]; +b2; DMA out.
Host gathers per-core [3, 8] outputs and undoes the batch assignment.
"""

import numpy as np

B, S, H = 64, 512, 1024
HID = 512
EPS = 1e-5
NCORES = 8
BPC = B // NCORES   # batches per core
KC = 3 * H // 128   # 24 contraction chunks for mm2
HC = H // 128       # 8 h-chunks per embedding
NMC = 2 * BPC       # mm1 psum partitions: 2 spans x 8 slots

BERT_PIECE = 3      # bert chunks per DMA piece
# W1 k-chunk pieces, in mm2 consumption order: pron (16..23) then e0/e1;
# small last piece keeps the post-DMA matmul tail short.
W1_PIECES = [(16, 24), (0, 8), (8, 14), (14, 16)]
MM2_ORDER = list(range(16, 24)) + list(range(0, 16))

# Test-harness hooks (harness calls kernel() with TRACE=False default).
TRACE = False
LAST_RESULT = None

_PROGRAM_CACHE: dict = {}


def _build_program(nch: int):
    """Build + compile the SPMD Bass program for nch 128-row bert chunks."""
    import concourse.bacc as bacc
    import concourse.tile as tile
    import concourse.mybir as mybir
    from concourse.bass import MemorySpace
    from concourse.masks import make_identity

    f32 = mybir.dt.float32
    bf16 = mybir.dt.bfloat16

    nc = bacc.Bacc("TRN2", target_bir_lowering=False, debug=False,
                   num_devices=NCORES)

    # cbf16: [mask (nch*16) | pronT (64)] per partition.
    # cf32:  [sfac (1) | bnbB (32) | w2 (12) | b2 (1)] per partition.
    cbf_d = nc.dram_tensor("cbf", [128, nch * NMC + HC * BPC], bf16,
                           kind="ExternalInput").ap()
    cf32_d = nc.dram_tensor("cf32", [128, 46], f32,
                            kind="ExternalInput").ap()
    bert_d = nc.dram_tensor("bertw", [nch * 128, H], bf16,
                            kind="ExternalInput").ap()
    w1_d = nc.dram_tensor("w1x", [128, KC, HID], bf16,
                          kind="ExternalInput").ap()
    out_d = nc.dram_tensor("out", [3, BPC], f32, kind="ExternalOutput").ap()

    with tile.TileContext(nc) as tc:
        with (
            tc.tile_pool(name="consts", bufs=1) as consts,
            tc.tile_pool(name="bert_pool", bufs=1) as bert_pool,
            tc.tile_pool(name="w1_pool", bufs=1) as w1_pool,
            tc.tile_pool(name="head", bufs=1) as head,
            tc.tile_pool(name="ps_mm1", bufs=1, space=MemorySpace.PSUM) as ps_mm1,
            tc.tile_pool(name="ps_tr", bufs=2, space=MemorySpace.PSUM) as ps_tr,
            tc.tile_pool(name="ps_mm2", bufs=1, space=MemorySpace.PSUM) as ps_mm2,
            tc.tile_pool(name="ps_mm3", bufs=1, space=MemorySpace.PSUM) as ps_mm3,
        ):
            # --- everything on the SP HWDGE ring, consts FIRST so mm1's
            # masks are never starved behind the bert stream ---
            cbf_t = consts.tile([128, nch * NMC + HC * BPC], bf16)
            nc.sync.dma_start(out=cbf_t, in_=cbf_d)
            cf32_t = consts.tile([128, 46], f32)
            nc.sync.dma_start(out=cf32_t, in_=cf32_d)
            mask_t = cbf_t[:, 0:nch * NMC].rearrange(
                "p (c m) -> p c m", c=nch)
            pron_t = cbf_t[:, nch * NMC:].rearrange(
                "p (k s) -> p k s", k=HC)
            sfac_t = cf32_t[0:NMC, 0:1]
            bnbB_t = cf32_t[:, 1:33].rearrange("p (m s) -> p m s", m=4)
            w2_t = cf32_t[:, 33:45].rearrange("p (m j) -> p m j", m=4)
            b2_t = cf32_t[0:3, 45:46]
            idt = consts.tile([NMC, NMC], f32)
            make_identity(nc, idt)

            # --- bulk streams: bert FIRST, W1 LAST ---
            # ~1.1MB pieces: big enough for near-line-rate, small enough
            # that mm1 stays pipelined behind the stream.
            bert_t = bert_pool.tile([128, nch, H], bf16)
            bert_src = bert_d.rearrange("(c p) h -> p c h", p=128)
            npieces = 4
            bounds = [round(i * nch / npieces) for i in range(npieces + 1)]
            for c0, c1 in zip(bounds[:-1], bounds[1:]):
                if c1 > c0:
                    nc.sync.dma_start(out=bert_t[:, c0:c1, :],
                                      in_=bert_src[:, c0:c1, :])
            w1_t = w1_pool.tile([128, KC, HID], bf16)
            for k0, k1 in W1_PIECES:
                nc.sync.dma_start(out=w1_t[:, k0:k1, :],
                                  in_=w1_d[:, k0:k1, :])

            # --- mm1: span sums, masks stationary, bert streaming ---
            x_sb = consts.tile([NMC, H], f32)
            ps_lo = ps_mm1.tile([NMC, 512], f32, tag="lo")
            ps_hi = ps_mm1.tile([NMC, 512], f32, tag="hi")
            for ch in range(nch):
                nc.tensor.matmul(ps_lo, mask_t[:, ch, :],
                                 bert_t[:, ch, 0:512],
                                 start=(ch == 0), stop=(ch == nch - 1))
                nc.tensor.matmul(ps_hi, mask_t[:, ch, :],
                                 bert_t[:, ch, 512:H],
                                 start=(ch == 0), stop=(ch == nch - 1))

            # --- mm2 part 1: pron kcs (no transpose dependency) ---
            ph = ps_mm2.tile([BPC, HID], f32)
            for i, kc in enumerate(MM2_ORDER[:HC]):
                nc.tensor.matmul(ph, pron_t[:, kc - 2 * HC, :],
                                 w1_t[:, kc, :], start=(i == 0), stop=False)

            # --- scale by 1/span_len (fp32), transpose to xT, cast bf16 ---
            nc.vector.tensor_scalar_mul(x_sb[:, 0:512], ps_lo, sfac_t)
            nc.vector.tensor_scalar_mul(x_sb[:, 512:H], ps_hi, sfac_t)
            xT_t = consts.tile([128, 2 * HC, BPC], bf16)
            for hc in range(HC):
                pht = ps_tr.tile([128, NMC], f32, tag="pht")
                nc.tensor.transpose(
                    pht, x_sb[:, hc * 128:(hc + 1) * 128], idt)
                nc.vector.tensor_copy(xT_t[:, hc, :], pht[:, 0:BPC])
                nc.vector.tensor_copy(xT_t[:, HC + hc, :], pht[:, BPC:NMC])

            # --- mm2 part 2: span kcs ---
            for i, kc in enumerate(MM2_ORDER[HC:]):
                nc.tensor.matmul(ph, xT_t[:, kc, :], w1_t[:, kc, :],
                                 start=False, stop=(i == 2 * HC - 1))

            # --- head: transpose h (4x into one PSUM tile), then single
            # +BN-bias and LeakyReLU over [128, 4*8], then mm3 ---
            hs_t = head.tile([BPC, HID], f32)
            ot_ps = ps_mm3.tile([3, BPC], f32, tag="oT")
            phtH = ps_tr.tile([128, 4 * BPC], f32, tag="phtH")
            for mc in range(HID // 128):
                nc.vector.tensor_copy(hs_t[:, mc * 128:(mc + 1) * 128],
                                      ph[:, mc * 128:(mc + 1) * 128])
                nc.tensor.transpose(
                    phtH[:, mc * BPC:(mc + 1) * BPC],
                    hs_t[:, mc * 128:(mc + 1) * 128],
                    idt[0:BPC, 0:BPC])
            t_t = head.tile([128, 4, BPC], f32, tag="t_t")
            nc.vector.tensor_add(
                t_t, phtH.rearrange("p (m s) -> p m s", m=4), bnbB_t)
            y_t = head.tile([128, 4, BPC], f32, tag="y_t")
            # y = max(0.01 * t, t)
            nc.vector.scalar_tensor_tensor(
                y_t, t_t, 0.01, t_t,
                op0=mybir.AluOpType.mult, op1=mybir.AluOpType.max)
            for mc in range(HID // 128):
                nc.tensor.matmul(
                    ot_ps, w2_t[:, mc, :], y_t[:, mc, :],
                    start=(mc == 0), stop=(mc == HID // 128 - 1))

            o_t = head.tile([3, BPC], f32)
            nc.vector.tensor_scalar_add(o_t, ot_ps, b2_t)
            nc.scalar.dma_start(out=out_d, in_=o_t)

    nc.compile()
    return nc


def _assign_batches(offs):
    """Union-row counts + balanced assignment of 8 batches to each core.

    Constrained LPT: batches sorted by union size desc, each goes to the
    least-loaded core that still has a free slot.
    """
    urows = np.empty(B, dtype=np.int64)
    for b in range(B):
        a0, a1, b0, b1_, _ = (int(v) for v in offs[b])
        la = a1 - a0 + 1
        lb = b1_ - b0 + 1
        ov = max(0, min(a1, b1_) - max(a0, b0) + 1)
        urows[b] = la + lb - ov
    order = np.argsort(-urows, kind="stable")
    loads = np.zeros(NCORES, dtype=np.int64)
    counts = np.zeros(NCORES, dtype=np.int64)
    assign = [[] for _ in range(NCORES)]
    for b in order:
        free = np.where(counts < BPC)[0]
        c = free[np.argmin(loads[free])]
        assign[c].append(int(b))
        loads[c] += urows[b]
        counts[c] += 1
    nch = int(np.ceil(loads.max() / 128))
    return assign, nch


def _prep_core_inputs(bert_f32, offs, batches, nch, w1x, cf32):
    """Build the per-core input map for the given 8 global batch ids."""
    import ml_dtypes
    bf16 = ml_dtypes.bfloat16

    rows_l, slots_l = [], []
    sfac = np.zeros((128,), dtype=np.float32)
    for s, gb in enumerate(batches):
        a0, a1, b0, b1_, _ = (int(v) for v in offs[gb])
        rows = np.union1d(np.arange(a0, a1 + 1), np.arange(b0, b1_ + 1))
        rows_l.append(rows)
        slots_l.append(np.full(len(rows), s, dtype=np.int64))
        sfac[s] = 1.0 / (a1 - a0 + 1)
        sfac[BPC + s] = 1.0 / (b1_ - b0 + 1)
    rows_cat = np.concatenate(rows_l)
    slots_cat = np.concatenate(slots_l)
    gb_cat = np.asarray(batches)[slots_cat]
    R = len(rows_cat)

    bertw = np.zeros((nch * 128, H), dtype=bf16)
    bertw[:R] = bert_f32[gb_cat, rows_cat].astype(bf16)

    a0s = offs[np.asarray(batches), 0][slots_cat]
    a1s = offs[np.asarray(batches), 1][slots_cat]
    b0s = offs[np.asarray(batches), 2][slots_cat]
    b1s = offs[np.asarray(batches), 3][slots_cat]
    mA = (rows_cat >= a0s) & (rows_cat <= a1s)
    mB = (rows_cat >= b0s) & (rows_cat <= b1s)
    maskflat = np.zeros((nch * 128, NMC), dtype=np.float32)
    idx = np.arange(R)
    maskflat[idx, slots_cat] = mA
    maskflat[idx, BPC + slots_cat] = mB
    maskp = maskflat.reshape(nch, 128, NMC).transpose(1, 0, 2)

    prons = offs[np.asarray(batches), 4]
    pron_rows = bert_f32[np.asarray(batches), prons]  # [BPC, H]
    pronT = pron_rows.reshape(BPC, HC, 128).transpose(2, 1, 0)  # [128,HC,BPC]

    cbf = np.empty((128, nch * NMC + HC * BPC), dtype=bf16)
    cbf[:, 0:nch * NMC] = maskp.reshape(128, nch * NMC).astype(bf16)
    cbf[:, nch * NMC:] = pronT.reshape(128, HC * BPC).astype(bf16)

    cf = cf32.copy()
    cf[:, 0] = sfac

    return {"cbf": cbf, "cf32": cf, "bertw": bertw, "w1x": w1x}


def kernel(bert_outputs, offsets, W1, b1, gamma, beta, running_mean,
           running_var, W2, b2):
    import ml_dtypes

    bert_f32 = np.ascontiguousarray(np.asarray(bert_outputs, dtype=np.float32))
    offs = np.asarray(offsets).astype(np.int64)
    W1 = np.asarray(W1, dtype=np.float32)
    b1 = np.asarray(b1, dtype=np.float32)
    gamma = np.asarray(gamma, dtype=np.float32)
    beta = np.asarray(beta, dtype=np.float32)
    rm = np.asarray(running_mean, dtype=np.float32)
    rv = np.asarray(running_var, dtype=np.float32)
    W2 = np.asarray(W2, dtype=np.float32)
    b2 = np.asarray(b2, dtype=np.float32)

    # Fold BN eval-mode stats: bn(xW1 + b1) = x(W1*s) + ((b1 - mean)*s + beta)
    s = gamma / np.sqrt(rv + EPS)
    bias = (b1 - rm) * s + beta
    w1f = (W1 * s[None, :]).astype(np.float32)
    # [3H, HID] -> [128 p, kc = e*8 + hc, HID], row = e*1024 + hc*128 + p
    w1x = np.ascontiguousarray(
        w1f.reshape(3, HC, 128, HID).transpose(2, 0, 1, 3)
        .reshape(128, KC, HID)).astype(ml_dtypes.bfloat16)

    # packed fp32 consts: [sfac | bnbB (4*8, bias bcast over slots) |
    #                      w2 (12) | b2 (1)]
    cf32 = np.zeros((128, 46), dtype=np.float32)
    cf32[:, 1:33] = np.repeat(
        bias.reshape(HID // 128, 128).T[:, :, None], BPC, axis=2
    ).reshape(128, 32)
    cf32[:, 33:45] = W2.reshape(HID // 128, 128, 3).transpose(1, 0, 2) \
        .reshape(128, 12)
    cf32[0:3, 45] = b2

    assign, nch = _assign_batches(offs)

    if nch not in _PROGRAM_CACHE:
        _PROGRAM_CACHE[nch] = _build_program(nch)
    nc = _PROGRAM_CACHE[nch]

    in_maps = [
        _prep_core_inputs(bert_f32, offs, assign[c], nch, w1x, cf32)
        for c in range(NCORES)
    ]

    from concourse import bass_utils
    kwargs = {}
    if TRACE:
        kwargs = {"trace": True, "trace_cores": list(range(NCORES))}
    res = bass_utils.run_bass_kernel_spmd(nc, in_maps,
                                          core_ids=list(range(NCORES)),
                                          **kwargs)
    global LAST_RESULT
    LAST_RESULT = res

    out = np.empty((B, 3), dtype=np.float32)
    for c in range(NCORES):
        out[assign[c]] = res.results[c]["out"].T
    return out
